# revision 5
# baseline (speedup 1.0000x reference)
"""SSN calc-assoc Trainium2 kernel (nn_CalcAssoc_53145925321404).

Strategy
--------
Host: for each batch image, bucket-sort the 65536 pixels by their center
superpixel id m (index_map value), pad each bucket to a multiple of 16 so
every 16-pixel partition-group is bucket-uniform, and split slots across
2 cores per batch (8 cores total, B=4).

Device (per core, all math on device except integer index/layout prep and
the tiny 256-entry snorm table):
  - One fused GEMM per 128-pixel tile (288 tiles):
      lhsT = [pixel_feats; pixel_feats^2]  (squares computed on-device, DVE)
      rhs  = W = [spixel_feats; -0.5]      ([128, 256], fp32r)
      out[p, n] = dot(pix_p, s_n) - 0.5*||pix_p||^2
  - gpsimd ap_gather pulls each pixel's 9 neighbor columns (bucket-uniform
    per 16-partition group, so the group-shared index lists are exact).
  - Epilogue: dist = -2*out_gathered + table, where table = snorm[nidx] for
    valid neighbors and 1e16 for invalid ones (1e16 - small == 1e16 in f32,
    bit-exact with the reference's INVALID_DIST).
Host: inverse-permute the per-slot results back to [B, 9, H, W].
"""
import numpy as np
from contextlib import ExitStack

import concourse.bacc as bacc
import concourse.tile as tile
from concourse import mybir
from concourse.bass_utils import run_bass_kernel_spmd

# problem constants (hardcoded per harness contract)
B, C, H, W = 4, 64, 256, 256
HW = H * W
NW, NH = 16, 16
NSP = NW * NH            # 256 superpixels
INVALID = np.float32(1e16)

# kernel layout constants
NT = 288                 # 128-pixel tiles per core
CAP = NT * 128           # 36864 slot capacity per core
CHUNK = 32               # tiles per pipeline chunk
NCH = NT // CHUNK        # 9 chunks
KIDX = 9                 # gather indices per 16-pixel slot (all real)
GW = CHUNK * KIDX        # gather output width per chunk (384)

# 3x3 neighbor tables: k -> (dy, dx)
_OFFS = np.arange(9)
_DY = _OFFS // 3 - 1
_DX = _OFFS % 3 - 1

_nc_cache = {}


def _neighbor_tables():
    """col[k, j]: clipped neighbor spixel id; valid[k, j]: in-grid mask."""
    j = np.arange(NSP)
    gx = j % NW
    gy = j // NW
    nx = gx[None, :] + _DX[:, None]          # [9, 256]
    ny = gy[None, :] + _DY[:, None]
    valid = (nx >= 0) & (nx < NW) & (ny >= 0) & (ny < NH)
    col = np.clip(ny, 0, NH - 1) * NW + np.clip(nx, 0, NW - 1)
    return col.astype(np.int64), valid


def _build_bass(loop_r=None):
    """Compile the per-core program (shared by all 8 cores)."""
    key = loop_r
    if key in _nc_cache:
        return _nc_cache[key]
    nc = bacc.Bacc("TRN2", target_bir_lowering=False, debug=False, num_devices=8)
    x_in = nc.dram_tensor("x", [C, CAP], mybir.dt.float32, kind="ExternalInput").ap()
    w_in = nc.dram_tensor("w", [128, NSP], mybir.dt.float32, kind="ExternalInput").ap()
    i_in = nc.dram_tensor("i", [128, NCH, GW // 16], mybir.dt.int16, kind="ExternalInput").ap()
    t_in = nc.dram_tensor("t", [128, NCH, GW], mybir.dt.float32, kind="ExternalInput").ap()
    o_out = nc.dram_tensor("o", [128, NCH * GW], mybir.dt.float32, kind="ExternalOutput").ap()

    with tile.TileContext(nc) as tc, ExitStack() as ctx:
        const_pool = ctx.enter_context(tc.tile_pool(name="const", bufs=1))
        stat_pool = ctx.enter_context(tc.tile_pool(name="stat", bufs=3))
        src_pool = ctx.enter_context(tc.tile_pool(name="src", bufs=3))
        idx_pool = ctx.enter_context(tc.tile_pool(name="idx", bufs=1))
        tbl_pool = ctx.enter_context(tc.tile_pool(name="tbl", bufs=3))
        g_pool = ctx.enter_context(tc.tile_pool(name="g", bufs=4))
        out_pool = ctx.enter_context(tc.tile_pool(name="out", bufs=4))
        psum_pool = ctx.enter_context(tc.tile_pool(name="psum", bufs=4, space="PSUM"))

        wt = const_pool.tile([128, NSP], mybir.dt.float32r)
        nc.sync.dma_start(wt[:], w_in[:].bitcast(mybir.dt.float32r))
        idxt = idx_pool.tile([128, NCH, GW // 16], mybir.dt.int16)
        nc.sync.dma_start(idxt[:], i_in[:])

        def body(_iv):
            for ch in range(NCH):
                x_sl = x_in[:, ch * CHUNK * 128:(ch + 1) * CHUNK * 128]
                stat = stat_pool.tile([128, CHUNK * 128], mybir.dt.float32r)
                nc.sync.dma_start(stat[0:C, :], x_sl.bitcast(mybir.dt.float32r))
                nc.vector.tensor_mul(stat[C:128, :],
                                     stat[0:C, :].bitcast(mybir.dt.float32),
                                     stat[0:C, :].bitcast(mybir.dt.float32))
                tblt = tbl_pool.tile([128, GW], mybir.dt.float32)
                nc.sync.dma_start(tblt[:], t_in[:, ch, :])
                src = src_pool.tile([128, CHUNK * NSP], mybir.dt.float32)
                for sp in range(CHUNK // 4):
                    pt = psum_pool.tile([128, 4 * NSP], mybir.dt.float32)
                    for q in range(4):
                        s = 4 * sp + q
                        nc.tensor.matmul(pt[:, q * NSP:(q + 1) * NSP],
                                         stat[:, s * 128:(s + 1) * 128], wt[:],
                                         start=True, stop=True)
                    dst = src[:, 4 * sp * NSP:(4 * sp + 4) * NSP]
                    if sp % 8 < 5:
                        nc.vector.tensor_copy(dst, pt[:])
                    else:
                        nc.scalar.copy(dst, pt[:])
                g1 = g_pool.tile([128, GW], mybir.dt.float32)
                nc.gpsimd.ap_gather(g1[:], src[:], idxt[:, ch, :], channels=128,
                                    num_elems=CHUNK * NSP, d=1, num_idxs=GW)
                ot = out_pool.tile([128, GW], mybir.dt.float32)
                nc.vector.tensor_scalar(ot[:], g1[:], -2.0, None,
                                        op0=mybir.AluOpType.mult)
                nc.vector.tensor_add(ot[:], ot[:], tblt[:])
                nc.sync.dma_start(o_out[:, ch * GW:(ch + 1) * GW], ot[:])

        if loop_r is None:
            body(None)
        else:
            with tc.For_i(0, loop_r, 1) as iv:
                body(iv)
    nc.compile()
    _nc_cache[key] = nc
    return nc


def _prep_core_inputs(pixel_feats, spixel_feats, index_map):
    """Sort/bucket/pad on host; build the 8 per-core input maps plus the
    unsort metadata (slot -> original pixel id)."""
    col_tab, valid_tab = _neighbor_tables()
    in_maps = []
    meta = []
    for b in range(B):
        m = np.asarray(index_map[b]).reshape(-1).astype(np.int64)
        order = np.argsort(m, kind="stable")
        counts = np.bincount(m, minlength=NSP)
        pad_counts = ((counts + 15) // 16) * 16
        total = int(pad_counts.sum())
        # slot arrays for the whole batch
        slot_px = np.full(total, -1, dtype=np.int64)
        slot_bucket = np.repeat(np.arange(NSP), pad_counts)
        off_pad = np.concatenate([[0], np.cumsum(pad_counts)[:-1]])
        off_real = np.concatenate([[0], np.cumsum(counts)[:-1]])
        pos = off_pad[m[order]] + (np.arange(HW) - off_real[m[order]])
        slot_px[pos] = order
        # split across the batch's two cores at a 16-aligned point
        split = min(CAP, ((total // 2 + 15) // 16) * 16)
        assert split <= CAP and (total - split) <= CAP
        feats = np.asarray(pixel_feats[b]).reshape(C, HW)
        snorm = (np.asarray(spixel_feats[b]).astype(np.float64) ** 2).sum(0)
        snorm = snorm.astype(np.float32)
        w_full = np.concatenate(
            [np.asarray(spixel_feats[b]).astype(np.float32),
             np.full((C, NSP), -0.5, dtype=np.float32)], axis=0)
        for half, (lo, hi) in enumerate(((0, split), (split, total))):
            n = hi - lo
            spx = np.full(CAP, -1, dtype=np.int64)
            sbk = np.zeros(CAP, dtype=np.int64)
            spx[:n] = slot_px[lo:hi]
            sbk[:n] = slot_bucket[lo:hi]
            xs = np.zeros((C, CAP), dtype=np.float32)
            real = spx >= 0
            xs[:, real] = feats[:, spx[real]]
            # group-uniform bucket per (tile, group)
            gb = sbk.reshape(NT * 8, 16)
            assert (gb == gb[:, :1]).all(), "16-slot group not bucket-uniform"
            gbt = gb[:, 0].reshape(NT, 8)          # [tile, group] -> bucket
            # gather indices: output position j = s*KIDX + k  (s = tile%CHUNK)
            s_of_t = np.arange(NT) % CHUNK
            cols = col_tab[:, gbt]                  # [9, NT, 8]
            vals = (cols.transpose(1, 2, 0)
                    + (s_of_t * NSP)[:, None, None])        # [NT, 8, 9]
            # wrapped storage: idx j at [16g + j%16, ch, j//16]
            idx_arr = np.zeros((128, NCH, GW // 16), dtype=np.int16)
            jpos = np.arange(GW)
            for g in range(8):
                per_chunk = vals.reshape(NCH, CHUNK, 8, KIDX)[:, :, g, :]
                flat = per_chunk.reshape(NCH, GW)   # j = s*KIDX + k
                idx_arr[16 * g + (jpos % 16), :, jpos // 16] = flat.T.astype(np.int16)[jpos]
            # epilogue table [128, NCH, GW]
            tbl = np.full((128, NT, KIDX), INVALID, dtype=np.float32)
            vt = valid_tab[:, gbt]                  # [9, NT, 8]
            sn = snorm[cols]                        # [9, NT, 8]
            for g in range(8):
                t_slice = np.where(vt[:, :, g].T, sn[:, :, g].T, INVALID)  # [NT, 9]
                tbl[16 * g:16 * (g + 1), :, :] = t_slice[None, :, :]
            tbl = tbl.reshape(128, NCH, GW)
            in_maps.append({"x": xs, "w": w_full, "i": idx_arr, "t": tbl})
            meta.append((b, spx))
    return in_maps, meta


def kernel(pixel_feats, spixel_feats, index_map, _loop_r=None, _nc=None):
    in_maps, meta = _prep_core_inputs(pixel_feats, spixel_feats, index_map)
    nc = _nc if _nc is not None else _build_bass(_loop_r)
    res = run_bass_kernel_spmd(nc, in_maps, core_ids=list(range(8)))
    out = np.empty((B, 9, HW), dtype=np.float32)
    for (b, spx), r in zip(meta, res.results):
        o = r["o"]                                  # [128, NCH*GW]
        arr = o.reshape(128, NCH, CHUNK, KIDX).transpose(1, 2, 0, 3)
        arr = arr.reshape(CAP, KIDX)                # slot-major
        real = spx >= 0
        out[b][:, spx[real]] = arr[real].T
    return out.reshape(B, 9, H, W)


# revision 6
# speedup vs baseline: 1.9130x; 1.9130x over previous
"""SSN calc-assoc Trainium2 kernel (nn_CalcAssoc_53145925321404).

Strategy
--------
Host: for each batch image, bucket-sort the 65536 pixels by their center
superpixel id m (index_map value), pad each bucket to a multiple of 16 so
every 16-pixel partition-group is bucket-uniform, and split slots across
2 cores per batch (8 cores total, B=4).

Device (per core, all math on device except integer index/layout prep and
the tiny 256-entry snorm table):
  - One fused GEMM per 128-pixel tile (288 tiles):
      lhsT = [pixel_feats; pixel_feats^2]  (squares computed on-device, DVE)
      rhs  = W = [spixel_feats; -0.5]      ([128, 256], fp32r)
      out[p, n] = dot(pix_p, s_n) - 0.5*||pix_p||^2
  - gpsimd ap_gather pulls each pixel's 9 neighbor columns (bucket-uniform
    per 16-partition group, so the group-shared index lists are exact).
  - Epilogue: dist = -2*out_gathered + table, where table = snorm[nidx] for
    valid neighbors and 1e16 for invalid ones (1e16 - small == 1e16 in f32,
    bit-exact with the reference's INVALID_DIST).
Host: inverse-permute the per-slot results back to [B, 9, H, W].
"""
import numpy as np
from contextlib import ExitStack

import concourse.bacc as bacc
import concourse.tile as tile
from concourse import mybir
from concourse.bass_utils import run_bass_kernel_spmd

# problem constants (hardcoded per harness contract)
B, C, H, W = 4, 64, 256, 256
HW = H * W
NW, NH = 16, 16
NSP = NW * NH            # 256 superpixels
INVALID = np.float32(1e16)

# kernel layout constants
NT = 288                 # 128-pixel tiles per core
CAP = NT * 128           # 36864 slot capacity per core
CHUNK = 32               # tiles per pipeline chunk
NCH = NT // CHUNK        # 9 chunks
KIDX = 9                 # gather indices per 16-pixel slot (all real)
GW = CHUNK * KIDX        # gather output width per chunk (384)

# 3x3 neighbor tables: k -> (dy, dx)
_OFFS = np.arange(9)
_DY = _OFFS // 3 - 1
_DX = _OFFS % 3 - 1

_nc_cache = {}


def _neighbor_tables():
    """col[k, j]: clipped neighbor spixel id; valid[k, j]: in-grid mask."""
    j = np.arange(NSP)
    gx = j % NW
    gy = j // NW
    nx = gx[None, :] + _DX[:, None]          # [9, 256]
    ny = gy[None, :] + _DY[:, None]
    valid = (nx >= 0) & (nx < NW) & (ny >= 0) & (ny < NH)
    col = np.clip(ny, 0, NH - 1) * NW + np.clip(nx, 0, NW - 1)
    return col.astype(np.int64), valid


def _build_bass(loop_r=None):
    """Compile the per-core program (shared by all 8 cores)."""
    key = loop_r
    if key in _nc_cache:
        return _nc_cache[key]
    nc = bacc.Bacc("TRN2", target_bir_lowering=False, debug=False, num_devices=8)
    x_in = nc.dram_tensor("x", [C, CAP], mybir.dt.float32, kind="ExternalInput").ap()
    w_in = nc.dram_tensor("w", [128, NSP], mybir.dt.float32, kind="ExternalInput").ap()
    i_in = nc.dram_tensor("i", [128, NCH, GW // 16], mybir.dt.int16, kind="ExternalInput").ap()
    t_in = nc.dram_tensor("t", [128, NCH, GW], mybir.dt.float32, kind="ExternalInput").ap()
    o_out = nc.dram_tensor("o", [128, NCH * GW], mybir.dt.float32, kind="ExternalOutput").ap()

    with tile.TileContext(nc) as tc, ExitStack() as ctx:
        const_pool = ctx.enter_context(tc.tile_pool(name="const", bufs=1))
        stat_pool = ctx.enter_context(tc.tile_pool(name="stat", bufs=3))
        src_pool = ctx.enter_context(tc.tile_pool(name="src", bufs=3))
        idx_pool = ctx.enter_context(tc.tile_pool(name="idx", bufs=1))
        tbl_pool = ctx.enter_context(tc.tile_pool(name="tbl", bufs=3))
        g_pool = ctx.enter_context(tc.tile_pool(name="g", bufs=4))
        out_pool = ctx.enter_context(tc.tile_pool(name="out", bufs=4))
        psum_pool = ctx.enter_context(tc.tile_pool(name="psum", bufs=4, space="PSUM"))

        wt = const_pool.tile([128, NSP], mybir.dt.float32r)
        nc.sync.dma_start(wt[:], w_in[:].bitcast(mybir.dt.float32r))
        idxt = idx_pool.tile([128, NCH, GW // 16], mybir.dt.int16)
        nc.sync.dma_start(idxt[:], i_in[:])

        def body(_iv):
            for ch in range(NCH):
                x_sl = x_in[:, ch * CHUNK * 128:(ch + 1) * CHUNK * 128]
                stat = stat_pool.tile([128, CHUNK * 128], mybir.dt.float32r)
                nc.sync.dma_start(stat[0:C, :], x_sl.bitcast(mybir.dt.float32r))
                nc.vector.tensor_mul(stat[C:128, :],
                                     stat[0:C, :].bitcast(mybir.dt.float32),
                                     stat[0:C, :].bitcast(mybir.dt.float32))
                tblt = tbl_pool.tile([128, GW], mybir.dt.float32)
                nc.sync.dma_start(tblt[:], t_in[:, ch, :])
                src = src_pool.tile([128, CHUNK * NSP], mybir.dt.float32)
                for sp in range(CHUNK // 4):
                    pt = psum_pool.tile([128, 4 * NSP], mybir.dt.float32)
                    for q in range(4):
                        s = 4 * sp + q
                        nc.tensor.matmul(pt[:, q * NSP:(q + 1) * NSP],
                                         stat[:, s * 128:(s + 1) * 128], wt[:],
                                         start=True, stop=True)
                    dst = src[:, 4 * sp * NSP:(4 * sp + 4) * NSP]
                    if sp % 4 < 3:
                        nc.vector.tensor_copy(dst, pt[:])
                    else:
                        nc.scalar.copy(dst, pt[:])
                g1 = g_pool.tile([128, GW], mybir.dt.float32)
                nc.gpsimd.ap_gather(g1[:], src[:], idxt[:, ch, :], channels=128,
                                    num_elems=CHUNK * NSP, d=1, num_idxs=GW)
                ot = out_pool.tile([128, GW], mybir.dt.float32)
                nc.vector.tensor_scalar(ot[:], g1[:], -2.0, None,
                                        op0=mybir.AluOpType.mult)
                nc.vector.tensor_add(ot[:], ot[:], tblt[:])
                nc.sync.dma_start(o_out[:, ch * GW:(ch + 1) * GW], ot[:])

        if loop_r is None:
            body(None)
        else:
            with tc.For_i(0, loop_r, 1) as iv:
                body(iv)
    nc.compile()
    _nc_cache[key] = nc
    return nc


def _prep_core_inputs(pixel_feats, spixel_feats, index_map):
    """Sort/bucket/pad on host; build the 8 per-core input maps plus the
    unsort metadata (slot -> original pixel id)."""
    col_tab, valid_tab = _neighbor_tables()
    in_maps = []
    meta = []
    for b in range(B):
        m = np.asarray(index_map[b]).reshape(-1).astype(np.int64)
        order = np.argsort(m, kind="stable")
        counts = np.bincount(m, minlength=NSP)
        pad_counts = ((counts + 15) // 16) * 16
        total = int(pad_counts.sum())
        # slot arrays for the whole batch
        slot_px = np.full(total, -1, dtype=np.int64)
        slot_bucket = np.repeat(np.arange(NSP), pad_counts)
        off_pad = np.concatenate([[0], np.cumsum(pad_counts)[:-1]])
        off_real = np.concatenate([[0], np.cumsum(counts)[:-1]])
        pos = off_pad[m[order]] + (np.arange(HW) - off_real[m[order]])
        slot_px[pos] = order
        # split across the batch's two cores at a 16-aligned point
        split = min(CAP, ((total // 2 + 15) // 16) * 16)
        assert split <= CAP and (total - split) <= CAP
        feats = np.asarray(pixel_feats[b]).reshape(C, HW)
        snorm = (np.asarray(spixel_feats[b]).astype(np.float64) ** 2).sum(0)
        snorm = snorm.astype(np.float32)
        w_full = np.concatenate(
            [np.asarray(spixel_feats[b]).astype(np.float32),
             np.full((C, NSP), -0.5, dtype=np.float32)], axis=0)
        for half, (lo, hi) in enumerate(((0, split), (split, total))):
            n = hi - lo
            spx = np.full(CAP, -1, dtype=np.int64)
            sbk = np.zeros(CAP, dtype=np.int64)
            spx[:n] = slot_px[lo:hi]
            sbk[:n] = slot_bucket[lo:hi]
            xs = np.zeros((C, CAP), dtype=np.float32)
            real = spx >= 0
            xs[:, real] = feats[:, spx[real]]
            # group-uniform bucket per (tile, group)
            gb = sbk.reshape(NT * 8, 16)
            assert (gb == gb[:, :1]).all(), "16-slot group not bucket-uniform"
            gbt = gb[:, 0].reshape(NT, 8)          # [tile, group] -> bucket
            # gather indices: output position j = s*KIDX + k  (s = tile%CHUNK)
            s_of_t = np.arange(NT) % CHUNK
            cols = col_tab[:, gbt]                  # [9, NT, 8]
            vals = (cols.transpose(1, 2, 0)
                    + (s_of_t * NSP)[:, None, None])        # [NT, 8, 9]
            # wrapped storage: idx j at [16g + j%16, ch, j//16]
            idx_arr = np.zeros((128, NCH, GW // 16), dtype=np.int16)
            jpos = np.arange(GW)
            for g in range(8):
                per_chunk = vals.reshape(NCH, CHUNK, 8, KIDX)[:, :, g, :]
                flat = per_chunk.reshape(NCH, GW)   # j = s*KIDX + k
                idx_arr[16 * g + (jpos % 16), :, jpos // 16] = flat.T.astype(np.int16)[jpos]
            # epilogue table [128, NCH, GW]
            tbl = np.full((128, NT, KIDX), INVALID, dtype=np.float32)
            vt = valid_tab[:, gbt]                  # [9, NT, 8]
            sn = snorm[cols]                        # [9, NT, 8]
            for g in range(8):
                t_slice = np.where(vt[:, :, g].T, sn[:, :, g].T, INVALID)  # [NT, 9]
                tbl[16 * g:16 * (g + 1), :, :] = t_slice[None, :, :]
            tbl = tbl.reshape(128, NCH, GW)
            in_maps.append({"x": xs, "w": w_full, "i": idx_arr, "t": tbl})
            meta.append((b, spx))
    return in_maps, meta


def kernel(pixel_feats, spixel_feats, index_map, _loop_r=None, _nc=None):
    in_maps, meta = _prep_core_inputs(pixel_feats, spixel_feats, index_map)
    nc = _nc if _nc is not None else _build_bass(_loop_r)
    res = run_bass_kernel_spmd(nc, in_maps, core_ids=list(range(8)))
    out = np.empty((B, 9, HW), dtype=np.float32)
    for (b, spx), r in zip(meta, res.results):
        o = r["o"]                                  # [128, NCH*GW]
        arr = o.reshape(128, NCH, CHUNK, KIDX).transpose(1, 2, 0, 3)
        arr = arr.reshape(CAP, KIDX)                # slot-major
        real = spx >= 0
        out[b][:, spx[real]] = arr[real].T
    return out.reshape(B, 9, H, W)
